# revision 1
# baseline (speedup 1.0000x reference)
"""3-branch 2-layer GAT classifier on 8 Trainium2 NeuronCores (Bass/Tile).

Strategy (edge-cut sharding per the hint):
- Nodes (and their incoming edges) are sharded contiguously across the 8
  cores; each core owns N/8 destination nodes for both GAT layers.
- Layer 1 is gather-free: the host expands x rows into per-edge-slot order
  (integer indexing only), and the tensor engine computes each slot's
  [feat | el | er] row directly via per-slot matmuls against
  [W | W.al | W.ar], writing straight into SBUF. A trailing indicator row
  in the expanded input turns padding slots into el = -1e30 so they drop
  out of the edge softmax.
- Layer 2: each core computes its shard of the layer-2 node table from its
  aggregated h1 rows, shards are AllGathered (the halo exchange), and
  source rows are fetched with one batched dma_gather per tile (split into
  a low/high pair because its indices are signed 16-bit).
- Edge softmax runs on ACT (Identity-with-bias, Exp-with-accum) and DVE
  (max-trick leaky relu, reductions); the weighted aggregation is a fused
  scalar*tensor+tensor multiply-accumulate alternating DVE and GPSIMD.
- Readout: per-graph mean via an indicator-matrix matmul accumulated in
  PSUM, partials AllReduced, then the small MLP head replicated per core.

Host-side work is integer indexing / layout only; all floating-point math
runs on the NeuronCores.
"""

import os
import numpy as np
from contextlib import ExitStack

import concourse.bass as bass
import concourse.tile as tile
from concourse import bacc, mybir
from concourse import bass_utils

AF = mybir.ActivationFunctionType
ALU = mybir.AluOpType
F32 = mybir.dt.float32
I16 = mybir.dt.int16
SPLIT = 32768           # dma_gather int16 index reach

NC = 8
P = 128
NEG_BIG = -1.0e30

LAST_EXEC_NS = None
_CACHE = {}


# ----------------------------------------------------------------------------
# Host-side integer preprocessing
# ----------------------------------------------------------------------------

def _pack_idx16(flat):
    """Pack a flat gather-position list into the dma_gather int16 SBUF
    layout: value for position i sits at [i % 16, i // 16], replicated
    across the 8 groups of 16 partitions."""
    n = len(flat)
    assert n % 16 == 0
    arr = np.asarray(flat, np.int64).reshape(n // 16, 16).T  # [16, n/16]
    return np.tile(arr, (8, 1)).astype(np.int16)


def _preprocess(src, dst, gid, N):
    Ncore = N // NC
    TILES = (Ncore + P - 1) // P
    NT = TILES * P
    NTS = NT + 1                  # shard rows incl. trailing dummy

    deg = np.bincount(dst, minlength=N)

    eorder = np.argsort(dst, kind="stable")
    srcs_sorted = src[eorder].astype(np.int64)
    rowptr = np.zeros(N + 1, np.int64)
    rowptr[1:] = np.cumsum(deg)

    node_order = np.full((NC, NT), -1, np.int64)
    for c in range(NC):
        d = deg[c * Ncore:(c + 1) * Ncore]
        o = np.argsort(-d, kind="stable")
        node_order[c, :Ncore] = c * Ncore + o

    degp = np.zeros((NC, NT), np.int64)
    for c in range(NC):
        real = node_order[c] >= 0
        degp[c, real] = deg[node_order[c][real]]
    K_t = np.maximum(degp.reshape(NC, TILES, P).max(axis=(0, 2)), 1)
    S1 = int(K_t.sum())
    off_t = np.zeros(TILES + 1, np.int64)
    off_t[1:] = np.cumsum(K_t)

    pos2 = np.zeros(N, np.int64)
    for c in range(NC):
        real = node_order[c] >= 0
        pos2[node_order[c][real]] = c * NTS + np.nonzero(real)[0]

    T2ROWS = NC * NTS

    # Overlapping int16-reach windows over the layer-2 table. Every row is
    # inside >= 1 window; rows in overlaps are assigned to balance the
    # per-partition counts (the padding cost is the per-tile max count).
    if T2ROWS <= SPLIT:
        wbase = [0]
    else:
        span = T2ROWS - SPLIT
        wbase = [0, span // 2, span]
    NW = len(wbase)
    # one dummy (el=-inf) row per window: each core's shard ends with one
    dummies = []
    for b in wbase:
        d = None
        for c in range(NC):
            row = c * NTS + NT
            if b <= row < b + SPLIT:
                d = row
                break
        assert d is not None
        dummies.append(d)

    # layer-1 slot sources (per core), -1 = padding slot
    slot_src = np.full((NC, S1, P), -1, np.int64)
    # layer-2 window-assigned slots per (core, tile, partition, window)
    wslots = [[[[[] for _ in range(NW)] for _ in range(P)]
               for _ in range(TILES)] for _ in range(NC)]
    Mmat = np.zeros((NC, TILES, P, P), np.float32)
    scat = np.zeros((NC, P, 1), np.int32)

    for c in range(NC):
        g_lo = gid[c * Ncore]
        assert gid[(c + 1) * Ncore - 1] - g_lo + 1 <= P
        scat[c, :, 0] = g_lo + np.arange(P)
        for t in range(TILES):
            for p in range(P):
                n = node_order[c, t * P + p]
                if n < 0:
                    continue
                dn = deg[n]
                es = srcs_sorted[rowptr[n]:rowptr[n] + dn]
                slot_src[c, off_t[t]:off_t[t] + dn, p] = es
                Mmat[c, t, p, gid[n] - g_lo] = 1.0
                # balanced window assignment (forced singles first)
                ws = wslots[c][t][p]
                items = []
                for q in pos2[es]:
                    elig = [w for w in range(NW)
                            if wbase[w] <= q < wbase[w] + SPLIT]
                    items.append((len(elig), q, elig))
                items.sort(key=lambda x: x[0])
                for _, q, elig in items:
                    w = min(elig, key=lambda w: len(ws[w]))
                    ws[w].append(q - wbase[w])

    # per-tile per-window padded counts, common across cores
    nW = np.zeros((TILES, NW), np.int64)
    for t in range(TILES):
        for c in range(NC):
            for p in range(P):
                for w in range(NW):
                    nW[t, w] = max(nW[t, w], len(wslots[c][t][p][w]))
    nW[:, 0] = np.maximum(nW[:, 0], 1)
    offW = np.zeros((TILES + 1, NW), np.int64)
    offW[1:] = np.cumsum(nW, axis=0)
    CW = nW.sum(axis=0).astype(np.int64)      # columns per window

    idx2 = [np.zeros((NC, P, max(int(CW[w]), 1) * 8), np.int16)
            for w in range(NW)]
    for c in range(NC):
        for t in range(TILES):
            for w in range(NW):
                nw = int(nW[t, w])
                if nw == 0:
                    continue
                fa = np.full((nw, P), dummies[w] - wbase[w], np.int64)
                for p in range(P):
                    v = wslots[c][t][p][w]
                    fa[:len(v), p] = v
                idx2[w][c][:, int(offW[t, w]) * 8:int(offW[t + 1, w]) * 8] = \
                    _pack_idx16(fa.reshape(-1))

    GROWS = 640
    cnt = np.maximum(np.bincount(gid, minlength=GROWS).astype(np.float32), 1.0)

    return dict(
        Ncore=Ncore, TILES=TILES, NT=NT, NTS=NTS, K_t=K_t, S1=S1, off_t=off_t,
        T2ROWS=T2ROWS, NW=NW, wbase=wbase, nW=nW, offW=offW, CW=CW,
        GROWS=GROWS, node_order=node_order, slot_src=slot_src,
        idx2=idx2, Mmat=Mmat, scat=scat, cnt=cnt.reshape(GROWS, 1),
    )


# ----------------------------------------------------------------------------
# Bass program
# ----------------------------------------------------------------------------

def _build_program(N, F, Gn, C, pre):
    TILES, NT, NTS = pre["TILES"], pre["NT"], pre["NTS"]
    K_t, S1, off_t = pre["K_t"], pre["S1"], pre["off_t"]
    NW, wbase, nW, offW, CW = (pre["NW"], pre["wbase"], pre["nW"],
                               pre["offW"], pre["CW"])
    T2ROWS, GROWS = pre["T2ROWS"], pre["GROWS"]

    HF = 2 * F                  # 200
    RW = HF + 4                 # slot row: feat(200) el(2) er(2)
    ROW2 = 128                  # t2 row: feat2(100) el2(1) er2(1) pad
    SLOTS1 = S1 * P
    FI = F + 1                  # x rows + pad-indicator row
    BF16 = mybir.dt.bfloat16

    nc = bacc.Bacc("TRN2", target_bir_lowering=False, debug=False,
                   enable_asserts=False, num_devices=NC)

    xg = [nc.dram_tensor(f"xg{b}", [FI, SLOTS1], BF16, kind="ExternalInput")
          for b in range(3)]
    xo = [nc.dram_tensor(f"xo{b}", [FI, NT], BF16, kind="ExternalInput")
          for b in range(3)]
    W1 = nc.dram_tensor("W1", [F, HF], F32, kind="ExternalInput")
    al1 = nc.dram_tensor("al1", [2, F], F32, kind="ExternalInput")
    ar1 = nc.dram_tensor("ar1", [2, F], F32, kind="ExternalInput")
    b1 = nc.dram_tensor("b1", [HF], F32, kind="ExternalInput")
    W2 = nc.dram_tensor("W2", [HF, F], F32, kind="ExternalInput")
    al2 = nc.dram_tensor("al2", [1, F], F32, kind="ExternalInput")
    ar2 = nc.dram_tensor("ar2", [1, F], F32, kind="ExternalInput")
    b2 = nc.dram_tensor("b2", [F], F32, kind="ExternalInput")
    Wfc = nc.dram_tensor("Wfc", [3 * F, F], F32, kind="ExternalInput")
    bfc = nc.dram_tensor("bfc", [F], F32, kind="ExternalInput")
    Wcls = nc.dram_tensor("Wcls", [F, C], F32, kind="ExternalInput")
    bcls = nc.dram_tensor("bcls", [C], F32, kind="ExternalInput")
    idx2 = [nc.dram_tensor(f"idx2w{w}", [P, max(int(CW[w]), 1) * 8], I16,
                           kind="ExternalInput") for w in range(NW)]
    Mm = nc.dram_tensor("Mm", [TILES, P, P], F32, kind="ExternalInput")
    scat = nc.dram_tensor("scat", [P, 1], mybir.dt.int32, kind="ExternalInput")
    cnt = nc.dram_tensor("cnt", [GROWS, 1], F32, kind="ExternalInput")
    wrow = nc.dram_tensor("wrow", [1, RW], F32, kind="ExternalInput")
    out = nc.dram_tensor("out", [Gn, C], F32, kind="ExternalOutput")

    def bcast(handle, n, parts=P):
        ap = handle.ap()
        return bass.AP(tensor=ap.tensor, offset=0, ap=[[0, parts], [1, n]])

    with tile.TileContext(nc) as tc, ExitStack() as ctx:
        sing = ctx.enter_context(tc.tile_pool(name="sing", bufs=1))
        xp = ctx.enter_context(tc.tile_pool(name="xp", bufs=2))
        ep = ctx.enter_context(tc.tile_pool(name="ep", bufs=2))
        sm = ctx.enter_context(tc.tile_pool(name="sm", bufs=3))
        hp = ctx.enter_context(tc.tile_pool(name="hp", bufs=2))
        mp = ctx.enter_context(tc.tile_pool(name="mp", bufs=2))
        pt1 = ctx.enter_context(tc.tile_pool(name="pt1", bufs=3, space="PSUM"))
        ptp = ctx.enter_context(tc.tile_pool(name="ptp", bufs=2, space="PSUM"))
        pt2 = ctx.enter_context(tc.tile_pool(name="pt2", bufs=2, space="PSUM"))
        pme = ctx.enter_context(tc.tile_pool(name="pme", bufs=1, space="PSUM"))
        dp = ctx.enter_context(tc.tile_pool(name="dp", bufs=2, space="DRAM"))
        dp1 = ctx.enter_context(tc.tile_pool(name="dp1", bufs=1, space="DRAM"))

        # ---------------- constants ----------------
        # W1e: [W1 | W1.al1 | W1.ar1] with a trailing pad-indicator row that
        # pushes padding slots' el/er to -1e30.
        W1e = sing.tile([FI, RW], F32)
        nc.sync.dma_start(out=W1e[0:F, 0:HF], in_=W1[:, :])
        tmp = sing.tile([F, HF], F32)
        attb = sing.tile([F, HF], F32)
        nc.sync.dma_start(out=attb[:], in_=bcast(al1, HF, F))
        nc.vector.tensor_tensor(out=tmp[:], in0=W1e[0:F, 0:HF], in1=attb[:],
                                op=ALU.mult)
        nc.vector.tensor_reduce(out=W1e[0:F, HF:HF + 2],
                                in_=tmp[:].rearrange("p (h f) -> p h f", h=2),
                                axis=mybir.AxisListType.X, op=ALU.add)
        nc.sync.dma_start(out=attb[:], in_=bcast(ar1, HF, F))
        nc.vector.tensor_tensor(out=tmp[:], in0=W1e[0:F, 0:HF], in1=attb[:],
                                op=ALU.mult)
        nc.vector.tensor_reduce(out=W1e[0:F, HF + 2:HF + 4],
                                in_=tmp[:].rearrange("p (h f) -> p h f", h=2),
                                axis=mybir.AxisListType.X, op=ALU.add)
        nc.sync.dma_start(out=W1e[F:FI, :], in_=wrow[:, :])
        # bf16 copy of the extended weight for the slot matmuls
        W1eb = sing.tile([FI, RW], BF16)
        nc.scalar.activation(out=W1eb[:], in_=W1e[:], func=AF.Copy,
                             bias=0.0, scale=1.0)

        W2e = []
        tmp2 = sing.tile([F, F], F32)
        attb2 = sing.tile([F, F], F32)
        for j in range(2):
            w = sing.tile([F, F + 2], F32, tag=f"W2e{j}")
            nc.sync.dma_start(out=w[:, 0:F], in_=W2[j * F:(j + 1) * F, :])
            nc.sync.dma_start(out=attb2[:], in_=bcast(al2, F, F))
            nc.vector.tensor_tensor(out=tmp2[:], in0=w[:, 0:F], in1=attb2[:],
                                    op=ALU.mult)
            nc.vector.tensor_reduce(out=w[:, F:F + 1], in_=tmp2[:],
                                    axis=mybir.AxisListType.X, op=ALU.add)
            nc.sync.dma_start(out=attb2[:], in_=bcast(ar2, F, F))
            nc.vector.tensor_tensor(out=tmp2[:], in0=w[:, 0:F], in1=attb2[:],
                                    op=ALU.mult)
            nc.vector.tensor_reduce(out=w[:, F + 1:F + 2], in_=tmp2[:],
                                    axis=mybir.AxisListType.X, op=ALU.add)
            W2e.append(w)

        b1rep = sing.tile([P, HF], F32)
        nc.sync.dma_start(out=b1rep[:], in_=bcast(b1, HF))
        b2rep = sing.tile([P, F], F32)
        nc.sync.dma_start(out=b2rep[:], in_=bcast(b2, F))
        bfcrep = sing.tile([P, F], F32)
        nc.sync.dma_start(out=bfcrep[:], in_=bcast(bfc, F))
        bclsrep = sing.tile([P, C], F32)
        nc.sync.dma_start(out=bclsrep[:], in_=bcast(bcls, C))
        wfc_sb = sing.tile([F, 3 * F], F32)
        for j in range(3):
            nc.sync.dma_start(out=wfc_sb[:, j * F:(j + 1) * F],
                              in_=Wfc[j * F:(j + 1) * F, :])
        wcls_sb = sing.tile([F, C], F32)
        nc.sync.dma_start(out=wcls_sb[:], in_=Wcls[:, :])
        ident = sing.tile([P, P], F32)
        from concourse.masks import make_identity
        make_identity(nc, ident[:])

        idxsb = []
        for w in range(NW):
            iw = sing.tile([P, max(int(CW[w]), 1) * 8], I16, tag=f"idxw{w}")
            nc.sync.dma_start(out=iw[:], in_=idx2[w][:, :])
            idxsb.append(iw)
        scatsb = sing.tile([P, 1], mybir.dt.int32)
        nc.sync.dma_start(out=scatsb[:], in_=scat[:, :])
        drow2 = sing.tile([1, ROW2], F32)
        nc.vector.memset(drow2[:], 0.0)
        nc.vector.memset(drow2[0:1, F:F + 1], NEG_BIG)
        partial = sing.tile([P, 3 * F], F32)

        def mac_chain(kind, out_ap, feats, scals, init_ap=None):
            """out = init + sum_k scals[k] * feats[k] as one serial chain.
            kind='dve': fused scalar*tensor+tensor on DVE.
            kind='pool': broadcast-mult + add pairs on GPSIMD (it has no
            TensorScalarPtr), with a scratch tile per term."""
            n = out_ap.shape[-1]
            first = True
            if kind == "dve":
                if init_ap is not None:
                    nc.scalar.activation(out=out_ap, in_=init_ap,
                                         func=AF.Copy, bias=0.0, scale=1.0)
                    first = False
                for f, sc in zip(feats, scals):
                    nc.vector.scalar_tensor_tensor(
                        out=out_ap, in0=f, scalar=sc,
                        in1=(f if first else out_ap),
                        op0=ALU.mult, op1=(ALU.bypass if first else ALU.add))
                    first = False
            else:
                if init_ap is not None:
                    nc.gpsimd.tensor_copy(out=out_ap, in_=init_ap)
                    first = False
                for f, sc in zip(feats, scals):
                    if first:
                        nc.gpsimd.tensor_tensor(
                            out=out_ap, in0=f, in1=sc.to_broadcast([P, n]),
                            op=ALU.mult)
                        first = False
                    else:
                        tmp_g = sm.tile([P, n], F32, tag="macg")
                        nc.gpsimd.tensor_tensor(
                            out=tmp_g[:], in0=f, in1=sc.to_broadcast([P, n]),
                            op=ALU.mult)
                        nc.gpsimd.tensor_tensor(out=out_ap, in0=out_ap,
                                                in1=tmp_g[:], op=ALU.add)

        # ---------------- per-branch pipeline ----------------
        for b in range(3):
            t2s = dp.tile([NTS, ROW2], F32, tag="t2shard")
            t2f = dp.tile([T2ROWS, ROW2], F32, tag="t2full",
                          addr_space="Shared")

            # er tables for own (destination) nodes
            ertab = sing.tile([P, 2 * TILES], F32, tag=f"ertab{b}")
            er2tab = sing.tile([P, TILES], F32, tag=f"er2tab{b}")
            for t in range(TILES):
                xoc = xp.tile([FI, P], BF16, tag="xoc")
                nc.sync.dma_start(out=xoc[:], in_=xo[b][:, t * P:(t + 1) * P])
                pse = ptp.tile([P, P], F32, tag="ptp")
                nc.tensor.matmul(pse[:, 0:4], lhsT=xoc[:],
                                 rhs=W1eb[:, HF:HF + 4], start=True, stop=True)
                nc.scalar.activation(out=ertab[:, 2 * t:2 * t + 2],
                                     in_=pse[:, 2:4], func=AF.Copy,
                                     bias=0.0, scale=1.0)

            # --- layer-1 edge phase (gather-free) + layer-2 table shard ---
            for t in range(TILES):
                K = int(K_t[t])
                o = int(off_t[t])
                xgc = xp.tile([FI, K * P], BF16, tag="xgc")
                nc.sync.dma_start(out=xgc[:],
                                  in_=xg[b][:, o * P:(o + K) * P])
                G = ep.tile([P, K, RW], F32, tag="G1")
                for k0 in range(0, K, 2):
                    kw = min(2, K - k0)
                    ps = pt1.tile([P, 2 * RW], F32, tag="pt1")
                    for j in range(kw):
                        nc.tensor.matmul(
                            ps[:, j * RW:(j + 1) * RW],
                            lhsT=xgc[:, (k0 + j) * P:(k0 + j + 1) * P],
                            rhs=W1eb[:], start=True, stop=True,
                            skip_group_check=True)
                    nc.scalar.activation(
                        out=G[:, k0:k0 + kw, :].rearrange("p k r -> p (k r)"),
                        in_=ps[:, 0:kw * RW], func=AF.Copy, bias=0.0,
                        scale=1.0)
                acc = hp.tile([P, HF], F32, tag="acc1")
                for h in range(2):
                    z = sm.tile([P, K], F32, tag="z")
                    nc.scalar.activation(out=z[:], in_=G[:, :, HF + h],
                                         func=AF.Identity,
                                         bias=ertab[:, 2 * t + h:2 * t + h + 1],
                                         scale=1.0)
                    e = sm.tile([P, K], F32, tag="e")
                    nc.vector.scalar_tensor_tensor(
                        out=e[:], in0=z[:], scalar=0.2, in1=z[:],
                        op0=ALU.mult, op1=ALU.max)
                    negm = sm.tile([P, 1], F32, tag="negm")
                    nc.vector.tensor_reduce(out=negm[:], in_=e[:],
                                            axis=mybir.AxisListType.X,
                                            op=ALU.max, negate=True)
                    a = sm.tile([P, K], F32, tag="a")
                    s = sm.tile([P, 1], F32, tag="s")
                    nc.scalar.activation(out=a[:], in_=e[:], func=AF.Exp,
                                         bias=negm[:, 0:1], scale=1.0,
                                         accum_out=s[:, 0:1])
                    rs = sm.tile([P, 1], F32, tag="rs")
                    nc.vector.reciprocal(out=rs[:], in_=s[:, 0:1])
                    al = sm.tile([P, K], F32, tag="al")
                    nc.scalar.activation(out=al[:], in_=a[:], func=AF.Identity,
                                         bias=0.0, scale=rs[:, 0:1])
                    # both head chains on DVE (GPSIMD is gather-bound)
                    mac_chain("dve",
                              acc[:, h * F:(h + 1) * F],
                              [G[:, k, h * F:(h + 1) * F] for k in range(K)],
                              [al[:, k:k + 1] for k in range(K)],
                              init_ap=b1rep[:, h * F:(h + 1) * F])
                # layer-2 table rows for this tile
                hTs = []
                for j in range(2):
                    tp = ptp.tile([P, P], F32, tag="ptp")
                    nc.tensor.transpose(tp[0:F, :], acc[:, j * F:(j + 1) * F],
                                        ident[:])
                    hT = hp.tile([F, P], F32, tag="hT")
                    nc.scalar.activation(out=hT[:], in_=tp[0:F, :],
                                         func=AF.Copy, bias=0.0, scale=1.0)
                    hTs.append(hT)
                ps2 = pt2.tile([P, F + 2], F32, tag="pt2")
                for j in range(2):
                    nc.tensor.matmul(ps2[:], lhsT=hTs[j][:], rhs=W2e[j][:],
                                     start=(j == 0), stop=(j == 1),
                                     skip_group_check=True)
                p2c = hp.tile([P, F + 2], F32, tag="t2c")
                nc.scalar.activation(out=p2c[:], in_=ps2[:], func=AF.Copy,
                                     bias=0.0, scale=1.0)
                nc.sync.dma_start(out=t2s[t * P:(t + 1) * P, 0:F + 2],
                                  in_=p2c[:])
                # own er2 straight from SBUF (avoids a scatter-read later)
                nc.scalar.activation(out=er2tab[:, t:t + 1],
                                     in_=p2c[:, F + 1:F + 2], func=AF.Copy,
                                     bias=0.0, scale=1.0)

            # --- halo exchange: AllGather layer-2 table shards ---
            nc.sync.dma_start(out=t2s[NT:NT + 1, :], in_=drow2[:])
            nc.gpsimd.collective_compute(
                "AllGather", ALU.bypass, replica_groups=[list(range(NC))],
                ins=[t2s[:, :]], outs=[t2f[:, :]])

            # --- layer-2 edge phase + graph-mean matmul ---
            pm = pme.tile([P, F], F32, tag="pme")
            for t in range(TILES):
                nws = [int(nW[t, w]) for w in range(NW)]
                nk = sum(nws)
                G2 = ep.tile([P, nk, ROW2], F32, tag="G2")
                # dma_gather tops out at 1024 indices per instruction
                g0 = 0
                for w in range(NW):
                    for c0 in range(0, nws[w], 8):
                        cw = min(8, nws[w] - c0)
                        a0 = (int(offW[t, w]) + c0) * 8
                        nc.gpsimd.dma_gather(
                            out_ap=G2[:, g0 + c0:g0 + c0 + cw, :],
                            in_ap=t2f[wbase[w]:, :] if wbase[w] else t2f[:, :],
                            idxs_ap=idxsb[w][:, a0:a0 + cw * 8],
                            num_idxs=cw * P, num_idxs_reg=cw * P,
                            elem_size=ROW2)
                    g0 += nws[w]
                acc2 = hp.tile([P, F], F32, tag="acc2")
                z = sm.tile([P, nk], F32, tag="z")
                nc.scalar.activation(out=z[:], in_=G2[:, :, F],
                                     func=AF.Identity,
                                     bias=er2tab[:, t:t + 1], scale=1.0)
                e = sm.tile([P, nk], F32, tag="e")
                nc.vector.scalar_tensor_tensor(
                    out=e[:], in0=z[:], scalar=0.2, in1=z[:],
                    op0=ALU.mult, op1=ALU.max)
                negm = sm.tile([P, 1], F32, tag="negm")
                nc.vector.tensor_reduce(out=negm[:], in_=e[:],
                                        axis=mybir.AxisListType.X,
                                        op=ALU.max, negate=True)
                a = sm.tile([P, nk], F32, tag="a")
                s = sm.tile([P, 1], F32, tag="s")
                nc.scalar.activation(out=a[:], in_=e[:], func=AF.Exp,
                                     bias=negm[:, 0:1], scale=1.0,
                                     accum_out=s[:, 0:1])
                rs = sm.tile([P, 1], F32, tag="rs")
                nc.vector.reciprocal(out=rs[:], in_=s[:, 0:1])
                al = sm.tile([P, nk], F32, tag="al")
                nc.scalar.activation(out=al[:], in_=a[:], func=AF.Identity,
                                     bias=0.0, scale=rs[:, 0:1])
                # two parallel partial chains (DVE: even slots incl. bias
                # init, GPSIMD: odd slots), then one combine add on DVE
                mac_chain("dve", acc2[:],
                          [G2[:, k, 0:F] for k in range(nk)],
                          [al[:, k:k + 1] for k in range(nk)],
                          init_ap=b2rep[:])
                Mt = mp.tile([P, P], F32, tag="M")
                nc.sync.dma_start(out=Mt[:], in_=Mm[t, :, :])
                nc.tensor.matmul(pm[:], lhsT=Mt[:], rhs=acc2[:],
                                 start=(t == 0), stop=(t == TILES - 1),
                                 skip_group_check=True)
            nc.scalar.activation(out=partial[:, b * F:(b + 1) * F],
                                 in_=pm[:], func=AF.Copy, bias=0.0, scale=1.0)

        # ---------------- readout ----------------
        pf = dp1.tile([GROWS, 3 * F], F32, tag="pf")
        rsum = dp1.tile([GROWS, 3 * F], F32, tag="rsum", addr_space="Shared")
        zsb = sing.tile([P, 3 * F], F32)
        nc.vector.memset(zsb[:], 0.0)
        for j in range(GROWS // P):
            nc.sync.dma_start(out=pf[j * P:(j + 1) * P, :], in_=zsb[:])
        nc.gpsimd.indirect_dma_start(
            out=pf[:, :],
            out_offset=bass.IndirectOffsetOnAxis(ap=scatsb[:, 0:1], axis=0),
            in_=partial[:], in_offset=None)
        nc.gpsimd.collective_compute(
            "AllReduce", ALU.add, replica_groups=[list(range(NC))],
            ins=[pf[:, :]], outs=[rsum[:, :]])

        GT = (Gn + P - 1) // P
        for gt in range(GT):
            rt = hp.tile([P, 3 * F], F32, tag="rt")
            nc.sync.dma_start(out=rt[:], in_=rsum[gt * P:(gt + 1) * P, :])
            cntt = sm.tile([P, 1], F32, tag="cntt")
            nc.sync.dma_start(out=cntt[:], in_=cnt[gt * P:(gt + 1) * P, :])
            rc = sm.tile([P, 1], F32, tag="rc")
            nc.vector.reciprocal(out=rc[:], in_=cntt[:, 0:1])
            rbar = hp.tile([P, 3 * F], F32, tag="rbar")
            nc.scalar.activation(out=rbar[:], in_=rt[:], func=AF.Identity,
                                 bias=0.0, scale=rc[:, 0:1])
            rTs = []
            for j in range(3):
                tp = ptp.tile([P, P], F32, tag="ptp")
                nc.tensor.transpose(tp[0:F, :], rbar[:, j * F:(j + 1) * F],
                                    ident[:])
                rT = hp.tile([F, P], F32, tag=f"rT{j}")
                nc.scalar.activation(out=rT[:], in_=tp[0:F, :], func=AF.Copy,
                                     bias=0.0, scale=1.0)
                rTs.append(rT)
            psfc = pt2.tile([P, F], F32, tag="pt2")
            for j in range(3):
                nc.tensor.matmul(psfc[:], lhsT=rTs[j][:],
                                 rhs=wfc_sb[:, j * F:(j + 1) * F],
                                 start=(j == 0), stop=(j == 2),
                                 skip_group_check=True)
            tfc = hp.tile([P, F], F32, tag="tfc")
            nc.vector.tensor_tensor(out=tfc[:], in0=psfc[:], in1=bfcrep[:],
                                    op=ALU.add)
            trel = hp.tile([P, F], F32, tag="trel")
            nc.scalar.activation(out=trel[:], in_=tfc[:], func=AF.Relu,
                                 bias=0.0, scale=1.0)
            tpc = ptp.tile([P, P], F32, tag="ptp")
            nc.tensor.transpose(tpc[0:F, :], trel[:], ident[:])
            tT = hp.tile([F, P], F32, tag="hT")
            nc.scalar.activation(out=tT[:], in_=tpc[0:F, :], func=AF.Copy,
                                 bias=0.0, scale=1.0)
            pscls = pt2.tile([P, C], F32, tag="pt2")
            nc.tensor.matmul(pscls[:], lhsT=tT[:], rhs=wcls_sb[:],
                             start=True, stop=True)
            ocls = hp.tile([P, C], F32, tag="ocls")
            nc.vector.tensor_tensor(out=ocls[:], in0=pscls[:], in1=bclsrep[:],
                                    op=ALU.add)
            rows = min(P, Gn - gt * P)
            nc.sync.dma_start(out=out[gt * P:gt * P + rows, :],
                              in_=ocls[0:rows, :])

    nc.compile()
    return nc


# ----------------------------------------------------------------------------
# Entry point
# ----------------------------------------------------------------------------

def kernel(**inputs):
    global LAST_EXEC_NS
    xs = [np.ascontiguousarray(np.asarray(inputs[k], np.float32))
          for k in ("x_pkt", "x_arv", "x_stat")]
    src = np.asarray(inputs["src"]).astype(np.int64)
    dst = np.asarray(inputs["dst"]).astype(np.int64)
    gid = np.asarray(inputs["graph_id"]).astype(np.int64)

    N, F = xs[0].shape
    Gn = 500
    C = int(np.asarray(inputs["bcls"]).shape[0])

    pre = _preprocess(src, dst, gid, N)

    key = (N, F, Gn, C, pre["S1"], tuple(pre["CW"].tolist()),
           tuple(pre["K_t"].tolist()), tuple(pre["nW"].reshape(-1).tolist()))
    if key not in _CACHE:
        _CACHE[key] = _build_program(N, F, Gn, C, pre)
    nc = _CACHE[key]

    S1, NT = pre["S1"], pre["NT"]
    SLOTS1 = S1 * P
    FI = F + 1

    common = {k: np.ascontiguousarray(np.asarray(inputs[k], np.float32))
              for k in ("W1", "al1", "ar1", "b1", "W2", "al2", "ar2", "b2",
                        "Wfc", "bfc", "Wcls", "bcls")}
    common["cnt"] = pre["cnt"]
    wr = np.zeros((1, 2 * F + 4), np.float32)
    wr[0, 2 * F:] = NEG_BIG
    common["wrow"] = wr

    import ml_dtypes
    BF = ml_dtypes.bfloat16
    xsb = [x.T.astype(BF) for x in xs]
    in_maps = []
    for c in range(NC):
        m = dict(common)
        ss = pre["slot_src"][c].reshape(-1)        # [S1*P], -1 = pad
        valid = ss >= 0
        for b in range(3):
            a = np.zeros((FI, SLOTS1), BF)
            a[:F, valid] = xsb[b][:, ss[valid]]
            a[F, ~valid] = 1.0
            m[f"xg{b}"] = a
            no = pre["node_order"][c]
            ov = no >= 0
            ao = np.zeros((FI, NT), BF)
            ao[:F, ov] = xsb[b][:, no[ov]]
            ao[F, ~ov] = 1.0
            m[f"xo{b}"] = ao
        for w in range(pre["NW"]):
            m[f"idx2w{w}"] = pre["idx2"][w][c]
        m["Mm"] = pre["Mmat"][c]
        m["scat"] = pre["scat"][c]
        in_maps.append(m)

    trace = os.environ.get("GAT_TRACE", "0") == "1"
    if trace:
        _install_trace_shim()
    r = bass_utils.run_bass_kernel_spmd(nc, in_maps, core_ids=list(range(NC)),
                                        trace=trace)
    LAST_EXEC_NS = r.exec_time_ns
    return np.asarray(r.results[0]["out"], np.float32)


def _install_trace_shim():
    import sys, types, contextlib, ctypes
    if "antenv.axon_hooks" in sys.modules:
        return
    so_path = "/opt/axon/libaxon_pjrt.so"
    lib = ctypes.CDLL(so_path)
    if not hasattr(lib, "axon_start_nrt_profile"):
        return
    lib.axon_start_nrt_profile.argtypes = [ctypes.POINTER(ctypes.c_int64),
                                           ctypes.c_size_t]
    lib.axon_start_nrt_profile.restype = ctypes.c_int64
    lib.axon_stop_nrt_profile.argtypes = [ctypes.c_char_p]
    lib.axon_stop_nrt_profile.restype = ctypes.c_int64

    @contextlib.contextmanager
    def _hook(output_dir, device_ids):
        import jax
        jax.devices()
        if device_ids:
            ids = (ctypes.c_int64 * len(device_ids))(*device_ids)
            rc = lib.axon_start_nrt_profile(ids, len(device_ids))
        else:
            rc = lib.axon_start_nrt_profile(None, 0)
        if rc != 0:
            raise RuntimeError(f"axon_start_nrt_profile rc={rc}")
        try:
            yield
        finally:
            n = lib.axon_stop_nrt_profile(str(output_dir).encode())
            print(f"profile: {n} file(s) written to {output_dir}")

    mod = types.ModuleType("antenv.axon_hooks")
    mod.get_axon_ntff_profile_hook = lambda: _hook
    mod.set_axon_ntff_profile_hook = lambda h: None
    sys.modules["antenv.axon_hooks"] = mod
    bass_utils.upload_artifacts = lambda tmpdir: f"file://{tmpdir}"



# revision 13
# speedup vs baseline: 1.5291x; 1.5291x over previous
"""3-branch 2-layer GAT classifier on 8 Trainium2 NeuronCores (Bass/Tile).

Strategy (edge-cut sharding per the hint):
- Nodes (and their incoming edges) are sharded contiguously across the 8
  cores; each core owns N/8 destination nodes for both GAT layers.
- Layer 1 is gather-free: the host expands x rows into per-edge-slot order
  (integer indexing only), and the tensor engine computes each slot's
  [feat | el | er] row directly via per-slot matmuls against
  [W | W.al | W.ar], writing straight into SBUF. A trailing indicator row
  in the expanded input turns padding slots into el = -1e30 so they drop
  out of the edge softmax.
- Layer 2: each core computes its shard of the layer-2 node table from its
  aggregated h1 rows, shards are AllGathered (the halo exchange), and
  source rows are fetched with one batched dma_gather per tile (split into
  a low/high pair because its indices are signed 16-bit).
- Edge softmax runs on ACT (Identity-with-bias, Exp-with-accum) and DVE
  (max-trick leaky relu, reductions); the weighted aggregation is a fused
  scalar*tensor+tensor multiply-accumulate alternating DVE and GPSIMD.
- Readout: per-graph mean via an indicator-matrix matmul accumulated in
  PSUM, partials AllReduced, then the small MLP head replicated per core.

Host-side work is integer indexing / layout only; all floating-point math
runs on the NeuronCores.
"""

import os
import numpy as np
from contextlib import ExitStack

import concourse.bass as bass
import concourse.tile as tile
from concourse import bacc, mybir
from concourse import bass_utils

AF = mybir.ActivationFunctionType
ALU = mybir.AluOpType
F32 = mybir.dt.float32
I16 = mybir.dt.int16
SPLIT = 32768           # dma_gather int16 index reach

NC = 8
P = 128
NEG_BIG = -1.0e30

LAST_EXEC_NS = None
_CACHE = {}


# ----------------------------------------------------------------------------
# Host-side integer preprocessing
# ----------------------------------------------------------------------------

def _pack_idx16(flat):
    """Pack a flat gather-position list into the dma_gather int16 SBUF
    layout: value for position i sits at [i % 16, i // 16], replicated
    across the 8 groups of 16 partitions."""
    n = len(flat)
    assert n % 16 == 0
    arr = np.asarray(flat, np.int64).reshape(n // 16, 16).T  # [16, n/16]
    return np.tile(arr, (8, 1)).astype(np.int16)


def _preprocess(src, dst, gid, N):
    Ncore = N // NC
    TILES = (Ncore + P - 1) // P
    NT = TILES * P
    NTS = NT + 1                  # shard rows incl. trailing dummy

    deg = np.bincount(dst, minlength=N)

    eorder = np.argsort(dst, kind="stable")
    srcs_sorted = src[eorder].astype(np.int64)
    rowptr = np.zeros(N + 1, np.int64)
    rowptr[1:] = np.cumsum(deg)

    node_order = np.full((NC, NT), -1, np.int64)
    for c in range(NC):
        d = deg[c * Ncore:(c + 1) * Ncore]
        o = np.argsort(-d, kind="stable")
        node_order[c, :Ncore] = c * Ncore + o

    degp = np.zeros((NC, NT), np.int64)
    for c in range(NC):
        real = node_order[c] >= 0
        degp[c, real] = deg[node_order[c][real]]
    K_t = np.maximum(degp.reshape(NC, TILES, P).max(axis=(0, 2)), 1)
    S1 = int(K_t.sum())
    off_t = np.zeros(TILES + 1, np.int64)
    off_t[1:] = np.cumsum(K_t)

    pos2 = np.zeros(N, np.int64)
    for c in range(NC):
        real = node_order[c] >= 0
        pos2[node_order[c][real]] = c * NTS + np.nonzero(real)[0]

    T2ROWS = NC * NTS

    # Overlapping int16-reach windows over the layer-2 table. Every row is
    # inside >= 1 window; rows in overlaps are assigned to balance the
    # per-partition counts (the padding cost is the per-tile max count).
    if T2ROWS <= SPLIT:
        wbase = [0]
    else:
        span = T2ROWS - SPLIT
        wbase = [0, span // 2, span]
    NW = len(wbase)
    # one dummy (el=-inf) row per window: each core's shard ends with one
    dummies = []
    for b in wbase:
        d = None
        for c in range(NC):
            row = c * NTS + NT
            if b <= row < b + SPLIT:
                d = row
                break
        assert d is not None
        dummies.append(d)

    # layer-1 slot sources (per core), -1 = padding slot
    slot_src = np.full((NC, S1, P), -1, np.int64)
    # layer-2 window-assigned slots per (core, tile, partition, window)
    wslots = [[[[[] for _ in range(NW)] for _ in range(P)]
               for _ in range(TILES)] for _ in range(NC)]
    Mmat = np.zeros((NC, TILES, P, P), np.float32)
    scat = np.zeros((NC, P, 1), np.int32)

    for c in range(NC):
        g_lo = gid[c * Ncore]
        assert gid[(c + 1) * Ncore - 1] - g_lo + 1 <= P
        scat[c, :, 0] = g_lo + np.arange(P)
        for t in range(TILES):
            for p in range(P):
                n = node_order[c, t * P + p]
                if n < 0:
                    continue
                dn = deg[n]
                es = srcs_sorted[rowptr[n]:rowptr[n] + dn]
                slot_src[c, off_t[t]:off_t[t] + dn, p] = es
                Mmat[c, t, p, gid[n] - g_lo] = 1.0
                # balanced window assignment (forced singles first)
                ws = wslots[c][t][p]
                items = []
                for q in pos2[es]:
                    elig = [w for w in range(NW)
                            if wbase[w] <= q < wbase[w] + SPLIT]
                    items.append((len(elig), q, elig))
                items.sort(key=lambda x: x[0])
                for _, q, elig in items:
                    w = min(elig, key=lambda w: len(ws[w]))
                    ws[w].append(q - wbase[w])

    # per-tile per-window padded counts, common across cores
    nW = np.zeros((TILES, NW), np.int64)
    for t in range(TILES):
        for c in range(NC):
            for p in range(P):
                for w in range(NW):
                    nW[t, w] = max(nW[t, w], len(wslots[c][t][p][w]))
    nW[:, 0] = np.maximum(nW[:, 0], 1)
    offW = np.zeros((TILES + 1, NW), np.int64)
    offW[1:] = np.cumsum(nW, axis=0)
    CW = nW.sum(axis=0).astype(np.int64)      # columns per window

    idx2 = [np.zeros((NC, P, max(int(CW[w]), 1) * 8), np.int16)
            for w in range(NW)]
    for c in range(NC):
        for t in range(TILES):
            for w in range(NW):
                nw = int(nW[t, w])
                if nw == 0:
                    continue
                fa = np.full((nw, P), dummies[w] - wbase[w], np.int64)
                for p in range(P):
                    v = wslots[c][t][p][w]
                    fa[:len(v), p] = v
                idx2[w][c][:, int(offW[t, w]) * 8:int(offW[t + 1, w]) * 8] = \
                    _pack_idx16(fa.reshape(-1))

    GROWS = 640
    cnt = np.maximum(np.bincount(gid, minlength=GROWS).astype(np.float32), 1.0)

    return dict(
        Ncore=Ncore, TILES=TILES, NT=NT, NTS=NTS, K_t=K_t, S1=S1, off_t=off_t,
        T2ROWS=T2ROWS, NW=NW, wbase=wbase, nW=nW, offW=offW, CW=CW,
        GROWS=GROWS, node_order=node_order, slot_src=slot_src,
        idx2=idx2, Mmat=Mmat, scat=scat, cnt=cnt.reshape(GROWS, 1),
    )


# ----------------------------------------------------------------------------
# Bass program
# ----------------------------------------------------------------------------

def _build_program(N, F, Gn, C, pre):
    TILES, NT, NTS = pre["TILES"], pre["NT"], pre["NTS"]
    K_t, S1, off_t = pre["K_t"], pre["S1"], pre["off_t"]
    NW, wbase, nW, offW, CW = (pre["NW"], pre["wbase"], pre["nW"],
                               pre["offW"], pre["CW"])
    T2ROWS, GROWS = pre["T2ROWS"], pre["GROWS"]

    HF = 2 * F                  # 200
    RW = HF + 4                 # slot row: feat(200) el(2) er(2)
    TROW = 384                  # t2 row: 3 branches x [feat2 el2 er2 pad->128]
    SLOTS1 = S1 * P
    FI = F + 1                  # x rows + pad-indicator row
    BF16 = mybir.dt.bfloat16

    nc = bacc.Bacc("TRN2", target_bir_lowering=False, debug=False,
                   enable_asserts=False, num_devices=NC)

    xg = [nc.dram_tensor(f"xg{b}", [FI, SLOTS1], BF16, kind="ExternalInput")
          for b in range(3)]
    xo = [nc.dram_tensor(f"xo{b}", [FI, NT], BF16, kind="ExternalInput")
          for b in range(3)]
    W1 = nc.dram_tensor("W1", [F, HF], F32, kind="ExternalInput")
    al1 = nc.dram_tensor("al1", [2, F], F32, kind="ExternalInput")
    ar1 = nc.dram_tensor("ar1", [2, F], F32, kind="ExternalInput")
    b1 = nc.dram_tensor("b1", [HF], F32, kind="ExternalInput")
    W2 = nc.dram_tensor("W2", [HF, F], F32, kind="ExternalInput")
    al2 = nc.dram_tensor("al2", [1, F], F32, kind="ExternalInput")
    ar2 = nc.dram_tensor("ar2", [1, F], F32, kind="ExternalInput")
    b2 = nc.dram_tensor("b2", [F], F32, kind="ExternalInput")
    Wfc = nc.dram_tensor("Wfc", [3 * F, F], F32, kind="ExternalInput")
    bfc = nc.dram_tensor("bfc", [F], F32, kind="ExternalInput")
    Wcls = nc.dram_tensor("Wcls", [F, C], F32, kind="ExternalInput")
    bcls = nc.dram_tensor("bcls", [C], F32, kind="ExternalInput")
    idx2 = [nc.dram_tensor(f"idx2w{w}", [P, max(int(CW[w]), 1) * 8], I16,
                           kind="ExternalInput") for w in range(NW)]
    Mm = nc.dram_tensor("Mm", [TILES, P, P], F32, kind="ExternalInput")
    scat = nc.dram_tensor("scat", [P, 1], mybir.dt.int32, kind="ExternalInput")
    cnt = nc.dram_tensor("cnt", [GROWS, 1], F32, kind="ExternalInput")
    wrow = nc.dram_tensor("wrow", [1, RW], F32, kind="ExternalInput")
    out = nc.dram_tensor("out", [Gn, C], F32, kind="ExternalOutput")
    dbg1 = nc.dram_tensor("dbg1", [NTS, TROW], mybir.dt.bfloat16,
                          kind="ExternalOutput")
    dbg2 = nc.dram_tensor("dbg2", [P, 3 * F], F32, kind="ExternalOutput")

    def bcast(handle, n, parts=P):
        ap = handle.ap()
        return bass.AP(tensor=ap.tensor, offset=0, ap=[[0, parts], [1, n]])

    with tile.TileContext(nc) as tc, ExitStack() as ctx:
        sing = ctx.enter_context(tc.tile_pool(name="sing", bufs=1))
        xp = ctx.enter_context(tc.tile_pool(name="xp", bufs=2))
        ep = ctx.enter_context(tc.tile_pool(name="ep", bufs=2))
        ep2 = ctx.enter_context(tc.tile_pool(name="ep2", bufs=2))
        g2p = ctx.enter_context(tc.tile_pool(name="g2p", bufs=3))
        ixp = ctx.enter_context(tc.tile_pool(name="ixp", bufs=3))
        sm = ctx.enter_context(tc.tile_pool(name="sm", bufs=3))
        hp = ctx.enter_context(tc.tile_pool(name="hp", bufs=2))
        mp = ctx.enter_context(tc.tile_pool(name="mp", bufs=2))
        pt1 = ctx.enter_context(tc.tile_pool(name="pt1", bufs=3, space="PSUM"))
        ptp = ctx.enter_context(tc.tile_pool(name="ptp", bufs=2, space="PSUM"))
        pt2 = ctx.enter_context(tc.tile_pool(name="pt2", bufs=2, space="PSUM"))
        pme = ctx.enter_context(tc.tile_pool(name="pme", bufs=1, space="PSUM"))
        dp1 = ctx.enter_context(tc.tile_pool(name="dp1", bufs=1, space="DRAM"))

        # ---------------- constants ----------------
        # W1e: [W1 | W1.al1 | W1.ar1] with a trailing pad-indicator row that
        # pushes padding slots' el/er to -1e30.
        W1e = sing.tile([FI, RW], F32)
        nc.sync.dma_start(out=W1e[0:F, 0:HF], in_=W1[:, :])
        tmp = sing.tile([F, HF], F32)
        attb = sing.tile([F, HF], F32)
        nc.sync.dma_start(out=attb[:], in_=bcast(al1, HF, F))
        nc.vector.tensor_tensor(out=tmp[:], in0=W1e[0:F, 0:HF], in1=attb[:],
                                op=ALU.mult)
        nc.vector.tensor_reduce(out=W1e[0:F, HF:HF + 2],
                                in_=tmp[:].rearrange("p (h f) -> p h f", h=2),
                                axis=mybir.AxisListType.X, op=ALU.add)
        nc.sync.dma_start(out=attb[:], in_=bcast(ar1, HF, F))
        nc.vector.tensor_tensor(out=tmp[:], in0=W1e[0:F, 0:HF], in1=attb[:],
                                op=ALU.mult)
        nc.vector.tensor_reduce(out=W1e[0:F, HF + 2:HF + 4],
                                in_=tmp[:].rearrange("p (h f) -> p h f", h=2),
                                axis=mybir.AxisListType.X, op=ALU.add)
        nc.sync.dma_start(out=W1e[F:FI, :], in_=wrow[:, :])
        # bf16 copy of the extended weight for the slot matmuls
        W1eb = sing.tile([FI, RW], BF16)
        nc.scalar.activation(out=W1eb[:], in_=W1e[:], func=AF.Copy,
                             bias=0.0, scale=1.0)

        W2e = []
        tmp2 = sing.tile([F, F], F32)
        attb2 = sing.tile([F, F], F32)
        for j in range(2):
            w = sing.tile([F, F + 2], F32, tag=f"W2e{j}")
            nc.sync.dma_start(out=w[:, 0:F], in_=W2[j * F:(j + 1) * F, :])
            nc.sync.dma_start(out=attb2[:], in_=bcast(al2, F, F))
            nc.vector.tensor_tensor(out=tmp2[:], in0=w[:, 0:F], in1=attb2[:],
                                    op=ALU.mult)
            nc.vector.tensor_reduce(out=w[:, F:F + 1], in_=tmp2[:],
                                    axis=mybir.AxisListType.X, op=ALU.add)
            nc.sync.dma_start(out=attb2[:], in_=bcast(ar2, F, F))
            nc.vector.tensor_tensor(out=tmp2[:], in0=w[:, 0:F], in1=attb2[:],
                                    op=ALU.mult)
            nc.vector.tensor_reduce(out=w[:, F + 1:F + 2], in_=tmp2[:],
                                    axis=mybir.AxisListType.X, op=ALU.add)
            W2e.append(w)

        b1rep = sing.tile([P, HF], F32)
        nc.sync.dma_start(out=b1rep[:], in_=bcast(b1, HF))
        b2rep = sing.tile([P, F], F32)
        nc.sync.dma_start(out=b2rep[:], in_=bcast(b2, F))
        bfcrep = sing.tile([P, F], F32)
        nc.sync.dma_start(out=bfcrep[:], in_=bcast(bfc, F))
        bclsrep = sing.tile([P, C], F32)
        nc.sync.dma_start(out=bclsrep[:], in_=bcast(bcls, C))
        wfc_sb = sing.tile([F, 3 * F], F32)
        for j in range(3):
            nc.sync.dma_start(out=wfc_sb[:, j * F:(j + 1) * F],
                              in_=Wfc[j * F:(j + 1) * F, :])
        wcls_sb = sing.tile([F, C], F32)
        nc.sync.dma_start(out=wcls_sb[:], in_=Wcls[:, :])
        ident = sing.tile([P, P], F32)
        from concourse.masks import make_identity
        make_identity(nc, ident[:])

        scatsb = sing.tile([P, 1], mybir.dt.int32)
        nc.sync.dma_start(out=scatsb[:], in_=scat[:, :])
        drow2 = sing.tile([1, TROW], BF16)
        nc.vector.memset(drow2[:], 0.0)
        for b in range(3):
            nc.vector.memset(drow2[0:1, b * 128 + F:b * 128 + F + 1], NEG_BIG)
        partial = sing.tile([P, 3 * F], F32)

        # ---------------- layer 1 (all 3 branches) ----------------
        # t2 node table rows are bf16, 3 branches interleaved at 128-col
        # stride: row = [b0: feat2(100) el2 er2 pad | b1: ... | b2: ...].
        # One gather per edge then serves all three branches.
        CW2 = 202                   # slot matmul cols: feat(200) el(2)
        t2all = dp1.tile([NTS, TROW], BF16, tag="t2all")
        t2f = dp1.tile([T2ROWS, TROW], BF16, tag="t2full",
                       addr_space="Shared")
        er2tabs = []
        for b in range(3):
            # er table for own (destination) nodes
            ertab = sing.tile([P, 2 * TILES], F32, tag=f"ertab{b}")
            er2tab = sing.tile([P, TILES], F32, tag=f"er2tab{b}")
            er2tabs.append(er2tab)
            for t in range(TILES):
                xoc = xp.tile([FI, P], BF16, tag="xoc")
                nc.sync.dma_start(out=xoc[:], in_=xo[b][:, t * P:(t + 1) * P])
                pse = ptp.tile([P, P], F32, tag="ptp")
                nc.tensor.matmul(pse[:, 0:4], lhsT=xoc[:],
                                 rhs=W1eb[:, HF:HF + 4], start=True, stop=True)
                nc.scalar.activation(out=ertab[:, 2 * t:2 * t + 2],
                                     in_=pse[:, 2:4], func=AF.Copy,
                                     bias=0.0, scale=1.0)

            # gather-free edge phase; weighted aggregation is one batched
            # broadcast-multiply + one strided reduce per head (not a chain)
            for t in range(TILES):
                K = int(K_t[t])
                o = int(off_t[t])
                xgc = xp.tile([FI, K * P], BF16, tag="xgc")
                nc.sync.dma_start(out=xgc[:],
                                  in_=xg[b][:, o * P:(o + K) * P])
                # G layout: [p, col, k] (k innermost) in bf16
                G = ep.tile([P, CW2 * K], BF16, tag="G1")
                Gv = G[:].rearrange("p (r k) -> p r k", k=K)
                for k0 in range(0, K, 2):
                    kw = min(2, K - k0)
                    ps = pt1.tile([P, 2 * RW], F32, tag="pt1")
                    for j in range(kw):
                        nc.tensor.matmul(
                            ps[:, j * RW:j * RW + CW2],
                            lhsT=xgc[:, (k0 + j) * P:(k0 + j + 1) * P],
                            rhs=W1eb[:, 0:CW2], start=True, stop=True,
                            skip_group_check=True)
                    nc.scalar.activation(
                        out=Gv[:, :, k0:k0 + kw],
                        in_=ps[:, 0:2 * RW].rearrange(
                            "p (k r) -> p r k", r=RW)[:, 0:CW2, 0:kw],
                        func=AF.Copy, bias=0.0, scale=1.0)
                acc = hp.tile([P, HF], F32, tag="acc1")
                for h in range(2):
                    z = sm.tile([P, K], F32, tag="z")
                    nc.scalar.activation(out=z[:], in_=Gv[:, HF + h, :],
                                         func=AF.Identity,
                                         bias=ertab[:, 2 * t + h:2 * t + h + 1],
                                         scale=1.0)
                    e = sm.tile([P, K], F32, tag="e")
                    nc.vector.scalar_tensor_tensor(
                        out=e[:], in0=z[:], scalar=0.2, in1=z[:],
                        op0=ALU.mult, op1=ALU.max)
                    negm = sm.tile([P, 1], F32, tag="negm")
                    nc.vector.tensor_reduce(out=negm[:], in_=e[:],
                                            axis=mybir.AxisListType.X,
                                            op=ALU.max, negate=True)
                    a = sm.tile([P, K], BF16, tag="a")
                    s = sm.tile([P, 1], F32, tag="s")
                    nc.scalar.activation(out=a[:], in_=e[:], func=AF.Exp,
                                         bias=negm[:, 0:1], scale=1.0,
                                         accum_out=s[:, 0:1])
                    rs = sm.tile([P, 1], F32, tag="rs")
                    nc.vector.reciprocal(out=rs[:], in_=s[:, 0:1])
                    # sf[p, f, k] = G[p, h*F+f, k] * a[p, k]; all APs have
                    # packed bf16 innermost dims so DVE runs in fast mode
                    sf = ep2.tile([P, F * K], BF16, tag="sf")
                    sfv = sf[:].rearrange("p (f k) -> p f k", k=K)
                    abc = a[:].rearrange("p (o k) -> p o k", o=1) \
                        .to_broadcast([P, F, K])
                    nc.vector.tensor_tensor(out=sfv,
                                            in0=Gv[:, h * F:(h + 1) * F, :],
                                            in1=abc, op=ALU.mult)
                    red = sm.tile([P, F], F32, tag="red")
                    nc.vector.tensor_reduce(out=red[:], in_=sfv,
                                            axis=mybir.AxisListType.X,
                                            op=ALU.add)
                    nc.vector.scalar_tensor_tensor(
                        out=acc[:, h * F:(h + 1) * F], in0=red[:],
                        scalar=rs[:, 0:1], in1=b1rep[:, h * F:(h + 1) * F],
                        op0=ALU.mult, op1=ALU.add)
                # layer-2 table rows for this tile
                hTs = []
                for j in range(2):
                    tp = ptp.tile([P, P], F32, tag="ptp")
                    nc.tensor.transpose(tp[0:F, :], acc[:, j * F:(j + 1) * F],
                                        ident[:])
                    hT = hp.tile([F, P], F32, tag="hT")
                    nc.scalar.activation(out=hT[:], in_=tp[0:F, :],
                                         func=AF.Copy, bias=0.0, scale=1.0)
                    hTs.append(hT)
                ps2 = pt2.tile([P, F + 2], F32, tag="pt2")
                for j in range(2):
                    nc.tensor.matmul(ps2[:], lhsT=hTs[j][:], rhs=W2e[j][:],
                                     start=(j == 0), stop=(j == 1),
                                     skip_group_check=True)
                stage = hp.tile([P, 128], BF16, tag="stage")
                nc.vector.memset(stage[:, F + 2:128], 0.0)
                nc.scalar.activation(out=stage[:, 0:F + 2], in_=ps2[:],
                                     func=AF.Copy, bias=0.0, scale=1.0)
                nc.sync.dma_start(
                    out=t2all[t * P:(t + 1) * P, b * 128:(b + 1) * 128],
                    in_=stage[:])
                # own er2 straight from PSUM (avoids a scatter-read later)
                nc.scalar.activation(out=er2tab[:, t:t + 1],
                                     in_=ps2[:, F + 1:F + 2], func=AF.Copy,
                                     bias=0.0, scale=1.0)

        # --- halo exchange: one AllGather of the interleaved table ---
        nc.sync.dma_start(out=t2all[NT:NT + 1, :], in_=drow2[:])
        nc.gpsimd.collective_compute(
            "AllGather", ALU.bypass, replica_groups=[list(range(NC))],
            ins=[t2all[:, :]], outs=[t2f[:, :]])

        # ---------------- layer 2 (all 3 branches per gather) ----------------
        pm = pme.tile([P, 3 * F], F32, tag="pme")
        for t in range(TILES):
            nws = [int(nW[t, w]) for w in range(NW)]
            nk = sum(nws)
            G2 = g2p.tile([P, nk * TROW], BF16, tag="G2")
            G2v = G2[:].rearrange("p (k e) -> p k e", e=TROW)
            # dma_gather tops out at 1024 indices per instruction
            g0 = 0
            for w in range(NW):
                if nws[w] == 0:
                    continue
                iw = ixp.tile([P, nws[w] * 8], I16, tag=f"ix{w}")
                nc.sync.dma_start(
                    out=iw[:],
                    in_=idx2[w][:, int(offW[t, w]) * 8:int(offW[t + 1, w]) * 8])
                for c0 in range(0, nws[w], 8):
                    cw = min(8, nws[w] - c0)
                    nc.gpsimd.dma_gather(
                        out_ap=G2v[:, g0 + c0:g0 + c0 + cw, :],
                        in_ap=t2f[wbase[w]:, :] if wbase[w] else t2f[:, :],
                        idxs_ap=iw[:, c0 * 8:(c0 + cw) * 8],
                        num_idxs=cw * P, num_idxs_reg=cw * P,
                        elem_size=TROW)
                g0 += nws[w]
            Mt = mp.tile([P, P], F32, tag="M")
            nc.sync.dma_start(out=Mt[:], in_=Mm[t, :, :])
            for b in range(3):
                z = sm.tile([P, nk], F32, tag="z")
                nc.scalar.activation(out=z[:], in_=G2v[:, :, b * 128 + F],
                                     func=AF.Identity,
                                     bias=er2tabs[b][:, t:t + 1], scale=1.0)
                e = sm.tile([P, nk], F32, tag="e")
                nc.vector.scalar_tensor_tensor(
                    out=e[:], in0=z[:], scalar=0.2, in1=z[:],
                    op0=ALU.mult, op1=ALU.max)
                negm = sm.tile([P, 1], F32, tag="negm")
                nc.vector.tensor_reduce(out=negm[:], in_=e[:],
                                        axis=mybir.AxisListType.X,
                                        op=ALU.max, negate=True)
                a = sm.tile([P, nk], BF16, tag="a")
                s = sm.tile([P, 1], F32, tag="s")
                nc.scalar.activation(out=a[:], in_=e[:], func=AF.Exp,
                                     bias=negm[:, 0:1], scale=1.0,
                                     accum_out=s[:, 0:1])
                rs = sm.tile([P, 1], F32, tag="rs")
                nc.vector.reciprocal(out=rs[:], in_=s[:, 0:1])
                # sf2 stored [p, f, k] (k packed) so the add-reduce over k
                # streams packed bf16
                sf2 = ep2.tile([P, F * nk], BF16, tag="sf2")
                sf2v = sf2[:].rearrange("p (f k) -> p k f", k=nk)
                abc = a[:].rearrange("p (k o) -> p k o", o=1) \
                    .to_broadcast([P, nk, F])
                nc.vector.tensor_tensor(
                    out=sf2v, in0=G2v[:, :, b * 128:b * 128 + F],
                    in1=abc, op=ALU.mult)
                red = sm.tile([P, F], F32, tag="red")
                nc.vector.tensor_reduce(
                    out=red[:],
                    in_=sf2[:].rearrange("p (f k) -> p f k", k=nk),
                    axis=mybir.AxisListType.X, op=ALU.add)
                acc2 = hp.tile([P, F], F32, tag="acc2")
                nc.vector.scalar_tensor_tensor(
                    out=acc2[:], in0=red[:], scalar=rs[:, 0:1],
                    in1=b2rep[:], op0=ALU.mult, op1=ALU.add)
                nc.tensor.matmul(pm[:, b * F:(b + 1) * F], lhsT=Mt[:],
                                 rhs=acc2[:],
                                 start=(t == 0 and b == 0),
                                 stop=(t == TILES - 1 and b == 2),
                                 skip_group_check=True)
        nc.scalar.activation(out=partial[:], in_=pm[:], func=AF.Copy,
                             bias=0.0, scale=1.0)
        nc.sync.dma_start(out=dbg1[:, :], in_=t2all[:, :])
        nc.sync.dma_start(out=dbg2[:, :], in_=partial[:])

        # ---------------- readout ----------------
        pf = dp1.tile([GROWS, 3 * F], F32, tag="pf")
        rsum = dp1.tile([GROWS, 3 * F], F32, tag="rsum", addr_space="Shared")
        zsb = sing.tile([P, 3 * F], F32)
        nc.vector.memset(zsb[:], 0.0)
        for j in range(GROWS // P):
            nc.sync.dma_start(out=pf[j * P:(j + 1) * P, :], in_=zsb[:])
        nc.gpsimd.indirect_dma_start(
            out=pf[:, :],
            out_offset=bass.IndirectOffsetOnAxis(ap=scatsb[:, 0:1], axis=0),
            in_=partial[:], in_offset=None)
        nc.gpsimd.collective_compute(
            "AllReduce", ALU.add, replica_groups=[list(range(NC))],
            ins=[pf[:, :]], outs=[rsum[:, :]])

        GT = (Gn + P - 1) // P
        for gt in range(GT):
            rt = hp.tile([P, 3 * F], F32, tag="rt")
            nc.sync.dma_start(out=rt[:], in_=rsum[gt * P:(gt + 1) * P, :])
            cntt = sm.tile([P, 1], F32, tag="cntt")
            nc.sync.dma_start(out=cntt[:], in_=cnt[gt * P:(gt + 1) * P, :])
            rc = sm.tile([P, 1], F32, tag="rc")
            nc.vector.reciprocal(out=rc[:], in_=cntt[:, 0:1])
            rbar = hp.tile([P, 3 * F], F32, tag="rbar")
            nc.scalar.activation(out=rbar[:], in_=rt[:], func=AF.Identity,
                                 bias=0.0, scale=rc[:, 0:1])
            rTs = []
            for j in range(3):
                tp = ptp.tile([P, P], F32, tag="ptp")
                nc.tensor.transpose(tp[0:F, :], rbar[:, j * F:(j + 1) * F],
                                    ident[:])
                rT = hp.tile([F, P], F32, tag=f"rT{j}")
                nc.scalar.activation(out=rT[:], in_=tp[0:F, :], func=AF.Copy,
                                     bias=0.0, scale=1.0)
                rTs.append(rT)
            psfc = pt2.tile([P, F], F32, tag="pt2")
            for j in range(3):
                nc.tensor.matmul(psfc[:], lhsT=rTs[j][:],
                                 rhs=wfc_sb[:, j * F:(j + 1) * F],
                                 start=(j == 0), stop=(j == 2),
                                 skip_group_check=True)
            tfc = hp.tile([P, F], F32, tag="tfc")
            nc.vector.tensor_tensor(out=tfc[:], in0=psfc[:], in1=bfcrep[:],
                                    op=ALU.add)
            trel = hp.tile([P, F], F32, tag="trel")
            nc.scalar.activation(out=trel[:], in_=tfc[:], func=AF.Relu,
                                 bias=0.0, scale=1.0)
            tpc = ptp.tile([P, P], F32, tag="ptp")
            nc.tensor.transpose(tpc[0:F, :], trel[:], ident[:])
            tT = hp.tile([F, P], F32, tag="hT")
            nc.scalar.activation(out=tT[:], in_=tpc[0:F, :], func=AF.Copy,
                                 bias=0.0, scale=1.0)
            pscls = pt2.tile([P, C], F32, tag="pt2")
            nc.tensor.matmul(pscls[:], lhsT=tT[:], rhs=wcls_sb[:],
                             start=True, stop=True)
            ocls = hp.tile([P, C], F32, tag="ocls")
            nc.vector.tensor_tensor(out=ocls[:], in0=pscls[:], in1=bclsrep[:],
                                    op=ALU.add)
            rows = min(P, Gn - gt * P)
            nc.sync.dma_start(out=out[gt * P:gt * P + rows, :],
                              in_=ocls[0:rows, :])

    nc.compile()
    return nc


# ----------------------------------------------------------------------------
# Entry point
# ----------------------------------------------------------------------------

def kernel(**inputs):
    global LAST_EXEC_NS
    xs = [np.ascontiguousarray(np.asarray(inputs[k], np.float32))
          for k in ("x_pkt", "x_arv", "x_stat")]
    src = np.asarray(inputs["src"]).astype(np.int64)
    dst = np.asarray(inputs["dst"]).astype(np.int64)
    gid = np.asarray(inputs["graph_id"]).astype(np.int64)

    N, F = xs[0].shape
    Gn = 500
    C = int(np.asarray(inputs["bcls"]).shape[0])

    pre = _preprocess(src, dst, gid, N)

    key = (N, F, Gn, C, pre["S1"], tuple(pre["CW"].tolist()),
           tuple(pre["K_t"].tolist()), tuple(pre["nW"].reshape(-1).tolist()))
    if key not in _CACHE:
        _CACHE[key] = _build_program(N, F, Gn, C, pre)
    nc = _CACHE[key]

    S1, NT = pre["S1"], pre["NT"]
    SLOTS1 = S1 * P
    FI = F + 1

    common = {k: np.ascontiguousarray(np.asarray(inputs[k], np.float32))
              for k in ("W1", "al1", "ar1", "b1", "W2", "al2", "ar2", "b2",
                        "Wfc", "bfc", "Wcls", "bcls")}
    common["cnt"] = pre["cnt"]
    wr = np.zeros((1, 2 * F + 4), np.float32)
    wr[0, 2 * F:] = NEG_BIG
    common["wrow"] = wr

    import ml_dtypes
    BF = ml_dtypes.bfloat16
    xsb = [x.T.astype(BF) for x in xs]
    in_maps = []
    for c in range(NC):
        m = dict(common)
        ss = pre["slot_src"][c].reshape(-1)        # [S1*P], -1 = pad
        valid = ss >= 0
        for b in range(3):
            a = np.zeros((FI, SLOTS1), BF)
            a[:F, valid] = xsb[b][:, ss[valid]]
            a[F, ~valid] = 1.0
            m[f"xg{b}"] = a
            no = pre["node_order"][c]
            ov = no >= 0
            ao = np.zeros((FI, NT), BF)
            ao[:F, ov] = xsb[b][:, no[ov]]
            ao[F, ~ov] = 1.0
            m[f"xo{b}"] = ao
        for w in range(pre["NW"]):
            m[f"idx2w{w}"] = pre["idx2"][w][c]
        m["Mm"] = pre["Mmat"][c]
        m["scat"] = pre["scat"][c]
        in_maps.append(m)

    trace = os.environ.get("GAT_TRACE", "0") == "1"
    if trace:
        _install_trace_shim()
    r = bass_utils.run_bass_kernel_spmd(nc, in_maps, core_ids=list(range(NC)),
                                        trace=trace)
    LAST_EXEC_NS = r.exec_time_ns
    global DBG
    if os.environ.get("GAT_DEBUG", "0") == "1":
        DBG = [(np.asarray(r.results[c]["dbg1"]), np.asarray(r.results[c]["dbg2"]))
               for c in range(NC)]
    return np.asarray(r.results[0]["out"], np.float32)


def _install_trace_shim():
    import sys, types, contextlib, ctypes
    if "antenv.axon_hooks" in sys.modules:
        return
    so_path = "/opt/axon/libaxon_pjrt.so"
    lib = ctypes.CDLL(so_path)
    if not hasattr(lib, "axon_start_nrt_profile"):
        return
    lib.axon_start_nrt_profile.argtypes = [ctypes.POINTER(ctypes.c_int64),
                                           ctypes.c_size_t]
    lib.axon_start_nrt_profile.restype = ctypes.c_int64
    lib.axon_stop_nrt_profile.argtypes = [ctypes.c_char_p]
    lib.axon_stop_nrt_profile.restype = ctypes.c_int64

    @contextlib.contextmanager
    def _hook(output_dir, device_ids):
        import jax
        jax.devices()
        if device_ids:
            ids = (ctypes.c_int64 * len(device_ids))(*device_ids)
            rc = lib.axon_start_nrt_profile(ids, len(device_ids))
        else:
            rc = lib.axon_start_nrt_profile(None, 0)
        if rc != 0:
            raise RuntimeError(f"axon_start_nrt_profile rc={rc}")
        try:
            yield
        finally:
            n = lib.axon_stop_nrt_profile(str(output_dir).encode())
            print(f"profile: {n} file(s) written to {output_dir}")

    mod = types.ModuleType("antenv.axon_hooks")
    mod.get_axon_ntff_profile_hook = lambda: _hook
    mod.set_axon_ntff_profile_hook = lambda h: None
    sys.modules["antenv.axon_hooks"] = mod
    bass_utils.upload_artifacts = lambda tmpdir: f"file://{tmpdir}"

